# revision 7
# baseline (speedup 1.0000x reference)
"""Trainium2 Bass kernel for nn_Attention_20298015441502 (sparse local attention).

Model: RMSNorm -> fused QKV -> (bug-faithful head-indexed RoPE) -> banded local
attention (window 256) -> out-proj -> residual.

Sharding (8 cores): data-parallel over batch (2) x tensor-parallel over heads
(4 groups of 4 heads). Core c = b*4+g computes batch b, heads [4g, 4g+4).
Out-proj is row-split; the host sums the 4 partials per batch and adds the
residual (that sum is the TP unshard step).

Host-side algebraic folds (exact reformulations):
 - The reference's RoPE angle depends on head index, not position, so the
   rotation is a constant orthogonal transform per head: fold into q/k weights.
 - norm_scale and the 1/sqrt(D) score scale fold into the weights too.
Device computes: rstd from x (ACT square + ones-matmul over partitions);
qkvT = W'^T x^T scaled by a PE-broadcast rstd row; per 128-key tile, transposed
scores k^T q over its <=5 attending 128-query tiles; p = exp(scores) in bf16
(scores are O(4): no max subtraction) with triangular masks on the two edge
tiles; o^T plus softmax sums via an appended ones-row on v (bf16 matmuls);
normalize by the broadcast reciprocal sums; out^T = woutT^T o^T.
"""

import sys

sys.path.insert(0, "/opt/trn_rl_repo")

import numpy as np
import ml_dtypes

import concourse.bass as bass
import concourse.bacc as bacc
import concourse.mybir as mybir
from concourse.bass_utils import run_bass_kernel_spmd
from concourse.bass_interp import get_hw_module
from concourse.tile import TileContext

F32 = mybir.dt.float32
F32R = mybir.dt.float32r
BF16 = mybir.dt.bfloat16

SIZE = 1024
HEADS = 16
HEAD = 64
EPS = 1e-5
ROPE_BASE = 10000.0
B, S = 2, 4096
HL = 4            # heads per core
NT = S // 128     # 32 s-tiles
NSG = S // 512    # 8 s-groups
CT = SIZE // 128  # 8 c-tiles
NL = 3 * HL * HEAD // 128   # 6 local qkv n-tiles
VW = HL * (HEAD + 1)        # v||ones block width per s-tile (260)

_cached = {}


def _build_module():
    nc = bacc.Bacc("TRN2", target_bir_lowering=False, debug=False)
    AF = mybir.ActivationFunctionType

    xT_d = nc.dram_tensor("xT", [SIZE, S], F32R, kind="ExternalInput").ap()
    wqkvT_d = nc.dram_tensor("wqkvT", [SIZE, 3 * HL * HEAD], F32R, kind="ExternalInput").ap()
    woutT_d = nc.dram_tensor("woutT", [HL * HEAD, SIZE], F32R, kind="ExternalInput").ap()
    onescol_d = nc.dram_tensor("onescol", [128, 1], BF16, kind="ExternalInput").ap()
    onesrow_d = nc.dram_tensor("onesrow", [1, 128], F32R, kind="ExternalInput").ap()
    mlo_d = nc.dram_tensor("mlo", [128, 128], BF16, kind="ExternalInput").ap()
    mhi_d = nc.dram_tensor("mhi", [128, 128], BF16, kind="ExternalInput").ap()
    ident_d = nc.dram_tensor("ident", [128, 128], F32, kind="ExternalInput").ap()
    outT_d = nc.dram_tensor("outT", [SIZE, S], F32, kind="ExternalOutput").ap()
    sums_b = nc.dram_tensor("sums_bounce", [4 * NT, 128], F32)
    inv_b = nc.dram_tensor("inv_bounce", [4 * NT, 128], F32)

    with TileContext(nc) as tc:
        with tc.tile_pool(name="consts", bufs=1) as consts:
            onescol = consts.tile([128, 1], BF16)
            nc.sync.dma_start(onescol, onescol_d)
            onesrow = consts.tile([1, 128], F32R)
            nc.sync.dma_start(onesrow, onesrow_d)
            mlo = consts.tile([128, 128], BF16)
            nc.sync.dma_start(mlo, mlo_d)
            mhi = consts.tile([128, 128], BF16)
            nc.sync.dma_start(mhi, mhi_d)
            ident = consts.tile([128, 128], F32)
            nc.sync.dma_start(ident, ident_d)
            epst = consts.tile([1, 1], F32)
            nc.vector.memset(epst, EPS)

            # persistent activations: q/k (f32r), v||ones (bf16), sums
            qk_sb = [consts.tile([128, S], F32R, tag=f"qk{n}", name=f"qk{n}") for n in range(4)]
            v_sb = consts.tile([128, NT * VW], BF16)
            sums_sb = consts.tile([4 * NT, 128], F32)

            ones_ap = bass.AP(tensor=v_sb.tensor, offset=v_sb.offset + HEAD,
                              ap=[v_sb.ap[0], [HEAD + 1, NT * HL], [1, 1]])
            nc.vector.memset(ones_ap, 1.0)

            # ---------------- Phase 1: rstd + QKV ----------------
            with tc.tile_pool(name="vtp", bufs=1) as vtp:
                vT_sb = [vtp.tile([128, S], F32, tag=f"vt{n}", name=f"vt{n}") for n in range(2)]
                with tc.tile_pool(name="wq", bufs=1) as wqp, \
                     tc.tile_pool(name="xin", bufs=2) as xin, \
                     tc.tile_pool(name="sq", bufs=3) as sqp, \
                     tc.tile_pool(name="bcs", bufs=2) as bcsp, \
                     tc.tile_pool(name="ps_qkv", bufs=2, space="PSUM") as ps_qkv, \
                     tc.tile_pool(name="ps_ss", bufs=2, space="PSUM") as ps_ss, \
                     tc.tile_pool(name="ps_bc", bufs=2, space="PSUM") as ps_bc:
                    wq_sb = [wqp.tile([128, NL * 128], F32R, tag=f"w{c}", name=f"w{c}")
                             for c in range(CT)]
                    for c in range(CT):
                        nc.sync.dma_start(wq_sb[c], wqkvT_d[c * 128:(c + 1) * 128, :])

                    for sg in range(NSG):
                        ssl = slice(sg * 512, (sg + 1) * 512)
                        xts = []
                        for c in range(CT):
                            xt = xin.tile([128, 512], F32R, tag=f"x{c}")
                            nc.sync.dma_start(xt, xT_d[c * 128:(c + 1) * 128, ssl])
                            xts.append(xt)
                        ssps = ps_ss.tile([1, 512], F32)
                        for c in range(CT):
                            sq = sqp.tile([128, 512], BF16)
                            nc.scalar.activation(sq, xts[c].bitcast(F32), AF.Square)
                            nc.tensor.matmul(ssps, onescol, sq,
                                             start=(c == 0), stop=(c == CT - 1))
                        rstd_s = bcsp.tile([1, 512], F32, tag="rstd_s")
                        nc.scalar.activation(rstd_s, ssps, AF.Sqrt,
                                             bias=epst, scale=1.0 / SIZE)
                        rstd = bcsp.tile([1, 512], F32R, tag="rstd")
                        with nc.allow_low_precision("f32r rounding for PE broadcast"):
                            nc.vector.reciprocal(rstd, rstd_s)
                        bcps = ps_bc.tile([128, 512], F32)
                        nc.tensor.matmul(bcps, onesrow, rstd, start=True, stop=True)
                        bcsb = bcsp.tile([128, 512], F32, tag="bcsb")
                        nc.scalar.copy(bcsb, bcps)
                        for n in range(NL):
                            mm = ps_qkv.tile([128, 512], F32)
                            for c in range(CT):
                                nc.tensor.matmul(mm, wq_sb[c][:, n * 128:(n + 1) * 128],
                                                 xts[c], start=(c == 0),
                                                 stop=(c == CT - 1))
                            if n < 4:
                                nc.vector.tensor_mul(qk_sb[n][:, ssl], mm, bcsb)
                            else:
                                nc.vector.tensor_mul(vT_sb[n - 4][:, ssl], mm, bcsb)

                # ---- Phase 1.5: transpose v into (s, d) bf16 blocks ----
                with tc.tile_pool(name="ps_tr", bufs=4, space="PSUM") as ps_tr:
                    for vt in range(2):
                        for st in range(NT):
                            trp = ps_tr.tile([128, 128], F32)
                            nc.tensor.transpose(
                                trp, vT_sb[vt][:, st * 128:(st + 1) * 128], ident)
                            base = st * VW + vt * 2 * (HEAD + 1)
                            dst = bass.AP(tensor=v_sb.tensor,
                                          offset=v_sb.offset + base,
                                          ap=[v_sb.ap[0], [HEAD + 1, 2], [1, HEAD]])
                            nc.scalar.copy(dst, trp)

            # ---------------- Phase 2: banded attention ----------------
            with tc.tile_pool(name="atp", bufs=1) as atp:
                oT_sb = [atp.tile([128, S], F32, tag=f"osb{p}", name=f"osb{p}") for p in range(2)]
                ptiles = {}
                with tc.tile_pool(name="pt", bufs=6) as ptp, \
                     tc.tile_pool(name="stg", bufs=4) as stgp, \
                     tc.tile_pool(name="ps_sc", bufs=4, space="PSUM") as ps_sc, \
                     tc.tile_pool(name="ps_ot", bufs=1, space="PSUM") as ps_ot:
                    ot_ps = [None] * HL

                    def emit_o(qt):
                        qg, j = divmod(qt, 4)
                        klo2, khi2 = max(qt - 2, 0), min(qt + 2, NT - 1)
                        for h in range(HL):
                            if j == 0:
                                ot_ps[h] = ps_ot.tile([HEAD + 1, 512], F32, tag=f"ot{h}", name=f"ot{h}")
                            dst = ot_ps[h][:, j * 128:(j + 1) * 128]
                            for idx, k2 in enumerate(range(klo2, khi2 + 1)):
                                pt, qlo2 = ptiles[(k2, h)]
                                off = (qt - qlo2) * 128
                                nc.tensor.matmul(
                                    dst,
                                    v_sb[:, k2 * VW + h * (HEAD + 1):
                                         k2 * VW + (h + 1) * (HEAD + 1)],
                                    pt[:, off:off + 128],
                                    start=(idx == 0), stop=(k2 == khi2))
                            if j == 3:
                                gsl = slice(qg * 512, (qg + 1) * 512)
                                nc.scalar.copy(
                                    oT_sb[h // 2][(h % 2) * 64:(h % 2) * 64 + 64, gsl],
                                    ot_ps[h][0:HEAD, :])
                                stage = stgp.tile([1, 512], F32)
                                nc.scalar.copy(stage, ot_ps[h][HEAD:HEAD + 1, :])
                                dstap = bass.AP(tensor=sums_b,
                                                offset=(qg * 16 + h) * 128,
                                                ap=[[512, 4], [1, 128]])
                                nc.sync.dma_start(dstap, stage)

                    for kt in range(NT):
                        qlo, qhi = max(kt - 2, 0), min(kt + 2, NT - 1)
                        span = (qhi - qlo + 1) * 128
                        chunks = ([(0, 384), (384, span - 384)] if span > 512
                                  else [(0, span)])
                        for h in range(HL):
                            ktile = qk_sb[2 + h // 2][(h % 2) * 64:(h % 2) * 64 + 64,
                                                      kt * 128:(kt + 1) * 128]
                            pt = ptp.tile([128, 640], BF16, tag=f"pt{h}")
                            for (c0, w) in chunks:
                                sps = ps_sc.tile([128, 512], F32)
                                qtile = qk_sb[h // 2][
                                    (h % 2) * 64:(h % 2) * 64 + 64,
                                    qlo * 128 + c0: qlo * 128 + c0 + w]
                                nc.tensor.matmul(sps[:, :w], ktile, qtile,
                                                 start=True, stop=True)
                                nc.scalar.activation(pt[:, c0:c0 + w], sps[:, :w],
                                                     AF.Exp)
                            if kt >= 2:
                                nc.vector.tensor_mul(pt[:, 0:128], pt[:, 0:128], mlo)
                            if kt <= NT - 3:
                                nc.vector.tensor_mul(pt[:, span - 128:span],
                                                     pt[:, span - 128:span], mhi)
                            ptiles[(kt, h)] = (pt, qlo)
                        if kt >= 2:
                            emit_o(kt - 2)
                    emit_o(NT - 2)
                    emit_o(NT - 1)

                # ---------- Phase 2.5 + 3: normalize + out-proj ----------
                with tc.tile_pool(name="wo", bufs=1) as wop, \
                     tc.tile_pool(name="onrmp", bufs=1) as onrmp, \
                     tc.tile_pool(name="ibc", bufs=3) as ibcp, \
                     tc.tile_pool(name="ost", bufs=3) as ostp, \
                     tc.tile_pool(name="ps_op", bufs=4, space="PSUM") as ps_op:
                    wo_sb = [wop.tile([128, SIZE], F32R, tag=f"wo{k}", name=f"wo{k}")
                             for k in range(2)]
                    for k in range(2):
                        nc.sync.dma_start(wo_sb[k], woutT_d[k * 128:(k + 1) * 128, :])
                    onrm = [onrmp.tile([128, S], F32R, tag=f"onrm{p}", name=f"onrm{p}")
                            for p in range(2)]
                    invt = ibcp.tile([4 * NT, 128], F32, tag="invt")
                    nc.sync.dma_start(sums_sb, sums_b[:, :])
                    nc.vector.reciprocal(invt, sums_sb)
                    nc.sync.dma_start(inv_b[:, :], invt)
                    for pair in range(2):
                        for st in range(NT):
                            ibsb = ibcp.tile([128, 128], F32, tag="ibsb")
                            src_ap = bass.AP(
                                tensor=inv_b,
                                offset=(st * 4 + pair * 2) * 128,
                                ap=[[128, 2], [0, 64], [1, 128]])
                            nc.sync.dma_start(ibsb, src_ap)
                            nc.vector.tensor_mul(
                                onrm[pair][:, st * 128:(st + 1) * 128],
                                oT_sb[pair][:, st * 128:(st + 1) * 128], ibsb)
                    for ct in range(CT):
                        for sg2 in range(NSG):
                            ops = ps_op.tile([128, 512], F32)
                            s2 = slice(sg2 * 512, (sg2 + 1) * 512)
                            nc.tensor.matmul(ops, wo_sb[0][:, ct * 128:(ct + 1) * 128],
                                             onrm[0][:, s2], start=True, stop=False)
                            nc.tensor.matmul(ops, wo_sb[1][:, ct * 128:(ct + 1) * 128],
                                             onrm[1][:, s2], start=False, stop=True)
                            ot = ostp.tile([128, 512], F32)
                            nc.scalar.copy(ot, ops)
                            nc.sync.dma_start(outT_d[ct * 128:(ct + 1) * 128, s2], ot)

    nc.compile()
    nc.m = get_hw_module(nc.m)
    return nc


def _rope_cos_sin(n, d):
    inv_freq = 1.0 / (ROPE_BASE ** (np.arange(0, d, 2, dtype=np.float32) / d))
    freqs = np.arange(n, dtype=np.float32)[:, None] * inv_freq[None, :]
    emb = np.concatenate([freqs, freqs], axis=-1)
    return np.cos(emb).astype(np.float32), np.sin(emb).astype(np.float32)


def _prep_inputs(x, w_qkv, w_out, norm_scale):
    cos, sin = _rope_cos_sin(HEADS, HEAD)
    Wf = np.ascontiguousarray(w_qkv.reshape(3, HEADS, HEAD, SIZE)).astype(np.float32)
    d2 = HEAD // 2

    def rot(Wh, h):
        out = np.empty_like(Wh)
        out[:d2] = cos[h, :d2, None] * Wh[:d2] - sin[h, :d2, None] * Wh[d2:]
        out[d2:] = cos[h, d2:, None] * Wh[d2:] + sin[h, d2:, None] * Wh[:d2]
        return out

    scale = np.float32(1.0 / np.sqrt(HEAD))
    Wq = np.stack([rot(Wf[0, h], h) for h in range(HEADS)]) * scale
    Wk = np.stack([rot(Wf[1, h], h) for h in range(HEADS)])
    Wv = Wf[2]
    ns = norm_scale.astype(np.float32)[None, None, :]
    Wq = Wq * ns
    Wk = Wk * ns
    Wv = Wv * ns

    i = np.arange(128)
    mlo = (i[:, None] <= i[None, :]).astype(ml_dtypes.bfloat16)
    mhi = (i[:, None] >= i[None, :]).astype(ml_dtypes.bfloat16)

    in_maps = []
    for core in range(8):
        b, g = divmod(core, 4)
        hs = slice(HL * g, HL * g + HL)
        wq_l = Wq[hs].reshape(HL * HEAD, SIZE)
        wk_l = Wk[hs].reshape(HL * HEAD, SIZE)
        wv_l = Wv[hs].reshape(HL * HEAD, SIZE)
        wqkvT = np.ascontiguousarray(np.concatenate([wq_l, wk_l, wv_l], 0).T)
        woutT = np.ascontiguousarray(
            w_out[:, HL * HEAD * g: HL * HEAD * (g + 1)].T.astype(np.float32))
        in_maps.append({
            "xT": np.ascontiguousarray(x[b].T.astype(np.float32)),
            "wqkvT": wqkvT,
            "woutT": woutT,
            "onescol": np.ones((128, 1), ml_dtypes.bfloat16),
            "onesrow": np.ones((1, 128), np.float32),
            "mlo": mlo,
            "mhi": mhi,
            "ident": np.eye(128, dtype=np.float32),
        })
    return in_maps


def _run(in_maps, trace=False, **kw):
    if "nc" not in _cached:
        _cached["nc"] = _build_module()
    return run_bass_kernel_spmd(_cached["nc"], in_maps, core_ids=list(range(8)),
                                trace=trace, **kw)


def kernel(x, mask, w_qkv, w_out, norm_scale):
    x = np.asarray(x)
    in_maps = _prep_inputs(x, np.asarray(w_qkv), np.asarray(w_out),
                           np.asarray(norm_scale))
    res = _run(in_maps)
    out = np.empty((B, S, SIZE), np.float32)
    for b in range(B):
        acc = res.results[b * 4]["outT"].copy()
        for g in range(1, 4):
            acc += res.results[b * 4 + g]["outT"]
        out[b] = acc.T + x[b]
    return out


# revision 12
# speedup vs baseline: 1.0750x; 1.0750x over previous
"""Trainium2 Bass kernel for nn_Attention_20298015441502 (sparse local attention).

Model: RMSNorm -> fused QKV -> (bug-faithful head-indexed RoPE) -> banded local
attention (window 256) -> out-proj -> residual.

Sharding (8 cores): data-parallel over batch (2) x tensor-parallel over heads
(4 groups of 4 heads). Core c = b*4+g computes batch b, heads [4g, 4g+4).
Out-proj is row-split; the host sums the 4 partials per batch and adds the
residual (that sum is the TP unshard step).

Host-side algebraic folds (exact reformulations):
 - The reference's RoPE angle depends on head index, not position, so the
   rotation is a constant orthogonal transform per head: fold into q/k weights.
 - norm_scale and the 1/sqrt(D) score scale fold into the weights too.
Device computes: rstd from x (ACT square + ones-matmul over partitions);
qkvT = W'^T x^T scaled by a PE-broadcast rstd row; per 128-key tile, transposed
scores k^T q over its <=5 attending 128-query tiles; p = exp(scores) in bf16
(scores are O(4): no max subtraction) with triangular masks on the two edge
tiles; o^T plus softmax sums via an appended ones-row on v (bf16 matmuls);
normalize by the broadcast reciprocal sums; out^T = woutT^T o^T.
"""

import sys

sys.path.insert(0, "/opt/trn_rl_repo")

import numpy as np
import ml_dtypes

import concourse.bass as bass
import concourse.bacc as bacc
import concourse.mybir as mybir
from concourse.bass_utils import run_bass_kernel_spmd
from concourse.bass_interp import get_hw_module
from concourse.tile import TileContext

F32 = mybir.dt.float32
F32R = mybir.dt.float32r
BF16 = mybir.dt.bfloat16

SIZE = 1024
HEADS = 16
HEAD = 64
EPS = 1e-5
ROPE_BASE = 10000.0
B, S = 2, 4096
HL = 4            # heads per core
NT = S // 128     # 32 s-tiles
NSG = S // 512    # 8 s-groups
CT = SIZE // 128  # 8 c-tiles
NL = 3 * HL * HEAD // 128   # 6 local qkv n-tiles
VB = 80                     # padded per-head v block (64 v + 1 ones + pad), 32B-aligned
VW = HL * VB                # v block width per s-tile

_cached = {}


def _build_module():
    nc = bacc.Bacc("TRN2", target_bir_lowering=False, debug=False)
    AF = mybir.ActivationFunctionType

    xT_d = nc.dram_tensor("xT", [SIZE, S], BF16, kind="ExternalInput").ap()
    wqkvT_d = nc.dram_tensor("wqkvT", [SIZE, 3 * HL * HEAD], BF16, kind="ExternalInput").ap()
    woutT_d = nc.dram_tensor("woutT", [HL * HEAD, SIZE], BF16, kind="ExternalInput").ap()
    onescol_d = nc.dram_tensor("onescol", [128, 1], BF16, kind="ExternalInput").ap()
    onesrow_d = nc.dram_tensor("onesrow", [1, 128], F32R, kind="ExternalInput").ap()
    mlo_d = nc.dram_tensor("mlo", [128, 128], BF16, kind="ExternalInput").ap()
    mhi_d = nc.dram_tensor("mhi", [128, 128], BF16, kind="ExternalInput").ap()
    outT_d = nc.dram_tensor("outT", [SIZE, S], F32, kind="ExternalOutput").ap()
    dbg = {}
    if _cached.get("debug"):
        for nm, shp, dt_ in [("d_qk0", [128, S], BF16), ("d_vsb", [128, NT * VW], BF16),
                             ("d_osb0", [128, S], F32), ("d_onrm0", [128, S], BF16),
                             ("d_inv", [HL, S], F32)]:
            dbg[nm] = nc.dram_tensor(nm, shp, dt_, kind="ExternalOutput").ap()
    inv_b = nc.dram_tensor("inv_bounce", [HL, S], F32)

    with TileContext(nc) as tc:
        with tc.tile_pool(name="consts", bufs=1) as consts:
            onescol = consts.tile([128, 1], BF16)
            nc.sync.dma_start(onescol, onescol_d)
            onesrow = consts.tile([1, 128], F32R)
            nc.sync.dma_start(onesrow, onesrow_d)
            mlo = consts.tile([128, 128], BF16)
            nc.sync.dma_start(mlo, mlo_d)
            mhi = consts.tile([128, 128], BF16)
            nc.sync.dma_start(mhi, mhi_d)
            epst = consts.tile([1, 1], F32)
            nc.vector.memset(epst, EPS)

            # persistent activations: q/k and v||ones in bf16
            qk_sb = [consts.tile([128, S], BF16, tag=f"qk{n}", name=f"qk{n}")
                     for n in range(4)]
            v_sb = consts.tile([128, NT * VW], BF16)
            ones_ap = bass.AP(tensor=v_sb.tensor, offset=v_sb.offset + HEAD,
                              ap=[v_sb.ap[0], [VB, NT * HL], [1, 1]])
            nc.vector.memset(ones_ap, 1.0)

            # ---------------- Phase 1: rstd + QKV (+ v transpose) ----------------
            with tc.tile_pool(name="wq", bufs=1) as wqp, \
                 tc.tile_pool(name="xin", bufs=2) as xin, \
                 tc.tile_pool(name="sq", bufs=3) as sqp, \
                 tc.tile_pool(name="bcs", bufs=2) as bcsp, \
                 tc.tile_pool(name="vt", bufs=2) as vtp, \
                 tc.tile_pool(name="ps_qkv", bufs=2, space="PSUM") as ps_qkv, \
                 tc.tile_pool(name="ps_ss", bufs=2, space="PSUM") as ps_ss, \
                 tc.tile_pool(name="ps_bc", bufs=2, space="PSUM") as ps_bc:
                wq_sb = [wqp.tile([128, NL * 128], BF16, tag=f"w{c}", name=f"w{c}")
                         for c in range(CT)]
                for c in range(CT):
                    eng = nc.sync if c % 2 == 0 else nc.gpsimd
                    eng.dma_start(wq_sb[c], wqkvT_d[c * 128:(c + 1) * 128, :])

                for sg in range(NSG):
                    ssl = slice(sg * 512, (sg + 1) * 512)
                    xts = []
                    for c in range(CT):
                        xt = xin.tile([128, 512], BF16, tag=f"x{c}")
                        eng = nc.sync if c % 2 == 0 else nc.gpsimd
                        eng.dma_start(xt, xT_d[c * 128:(c + 1) * 128, ssl])
                        xts.append(xt)
                    ssps = ps_ss.tile([1, 512], F32)
                    for c in range(CT):
                        sq = sqp.tile([128, 512], BF16)
                        nc.vector.tensor_mul(sq, xts[c], xts[c])
                        nc.tensor.matmul(ssps, onescol, sq,
                                         start=(c == 0), stop=(c == CT - 1))
                    # rstd = exp(-0.5*ln(ms+eps))
                    lnt = bcsp.tile([1, 512], F32, tag="lnt")
                    nc.scalar.activation(lnt, ssps, AF.Ln, bias=epst,
                                         scale=1.0 / SIZE)
                    rstd = bcsp.tile([1, 512], F32R, tag="rstd")
                    nc.scalar.activation(rstd, lnt, AF.Exp, scale=-0.5)
                    bcps = ps_bc.tile([128, 512], F32)
                    nc.tensor.matmul(bcps, onesrow, rstd, start=True, stop=True)
                    bcsb = bcsp.tile([128, 512], F32, tag="bcsb")
                    nc.scalar.copy(bcsb, bcps)
                    for n in range(NL):
                        mm = ps_qkv.tile([128, 512], F32)
                        for c in range(CT):
                            nc.tensor.matmul(mm, wq_sb[c][:, n * 128:(n + 1) * 128],
                                             xts[c], start=(c == 0),
                                             stop=(c == CT - 1))
                        if n < 4:
                            nc.vector.tensor_mul(qk_sb[n][:, ssl], mm, bcsb)
                        else:
                            vtt = vtp.tile([128, 512], BF16, tag=f"vtt{n-4}")
                            nc.vector.tensor_mul(vtt, mm, bcsb)
                            # DMA-transpose v into (s, d) blocks of v_sb
                            for u in range(4):
                                st = sg * 4 + u
                                for hh in range(2):
                                    h = (n - 4) * 2 + hh
                                    dst = v_sb[:, st * VW + h * VB:
                                               st * VW + h * VB + HEAD]
                                    nc.sync.dma_start_transpose(
                                        dst, vtt[hh * 64:hh * 64 + 64,
                                                 u * 128:(u + 1) * 128])

            # -------- Phase 2: banded attention + normalize + out-proj --------
            with tc.tile_pool(name="atp", bufs=1) as atp, \
                 tc.tile_pool(name="wo", bufs=1) as wop:
                oT_sb = [atp.tile([128, S], F32, tag=f"osb{p}", name=f"osb{p}")
                         for p in range(2)]
                onrm = [atp.tile([128, S], BF16, tag=f"onrm{p}", name=f"onrm{p}")
                        for p in range(2)]
                wo_sb = [wop.tile([128, SIZE], BF16, tag=f"wo{k}", name=f"wo{k}")
                         for k in range(2)]
                for k in range(2):
                    nc.gpsimd.dma_start(wo_sb[k], woutT_d[k * 128:(k + 1) * 128, :])

                ptiles = {}
                with tc.tile_pool(name="pt", bufs=9) as ptp, \
                     tc.tile_pool(name="stg", bufs=4) as stgp, \
                     tc.tile_pool(name="ibc", bufs=3) as ibcp, \
                     tc.tile_pool(name="ost", bufs=4) as ostp, \
                     tc.tile_pool(name="ps_sc", bufs=2, space="PSUM") as ps_sc, \
                     tc.tile_pool(name="ps_ot", bufs=1, space="PSUM") as ps_ot, \
                     tc.tile_pool(name="ps_op", bufs=2, space="PSUM") as ps_op:

                    def emit_group(qg):
                        # o-matmuls for q-tiles 4qg..4qg+3, k2-major so the
                        # stationary v block is reused across regions
                        t0, t3 = 4 * qg, 4 * qg + 3
                        klo_g, khi_g = max(t0 - 2, 0), min(t3 + 2, NT - 1)
                        ots = [ps_ot.tile([HEAD + 1, 512], F32, tag=f"ot{h}",
                                          name=f"ot{h}") for h in range(HL)]
                        # k2 = t0+1 attends the whole group (N=512): do it first
                        # with start=True since start clears the full psum bank.
                        k2s = [t0 + 1] + [k for k in range(klo_g, khi_g + 1)
                                          if k != t0 + 1]
                        for ki, k2 in enumerate(k2s):
                            ts = [t for t in range(t0, t3 + 1)
                                  if k2 - 2 <= t <= k2 + 2]
                            for h in range(HL):
                                pt, qlo2 = ptiles[(k2, h)]
                                vs = v_sb[:, k2 * VW + h * VB:
                                          k2 * VW + h * VB + HEAD + 1]
                                ja, jb = ts[0] - t0, ts[-1] - t0
                                off = (ts[0] - qlo2) * 128
                                nc.tensor.matmul(
                                    ots[h][:, ja * 128:(jb + 1) * 128],
                                    vs, pt[:, off:off + 128 * len(ts)],
                                    start=(ki == 0), stop=(ki == len(k2s) - 1),
                                    skip_group_check=True)
                        gsl = slice(qg * 512, (qg + 1) * 512)
                        for h in range(HL):
                            pair = h // 2
                            # softmax sums -> 1/sums via exp(-ln)
                            lns = stgp.tile([1, 512], F32, tag="lns")
                            nc.scalar.activation(lns, ots[h][HEAD:HEAD + 1, :], AF.Ln)
                            inv = stgp.tile([1, 512], F32, tag="inv")
                            nc.scalar.activation(inv, lns, AF.Exp, scale=-1.0)
                            nc.gpsimd.dma_start(inv_b[h, gsl], inv)
                            nc.scalar.copy(
                                oT_sb[pair][(h % 2) * 64:(h % 2) * 64 + 64, gsl],
                                ots[h][0:HEAD, :])
                        for pair in range(2):
                            ibsb = ibcp.tile([128, 512], F32, tag="ibsb")
                            src_ap = bass.AP(tensor=inv_b,
                                             offset=pair * 2 * S + qg * 512,
                                             ap=[[S, 2], [0, 64], [1, 512]])
                            nc.gpsimd.dma_start(ibsb, src_ap)
                            nc.vector.tensor_mul(onrm[pair][:, gsl],
                                                 oT_sb[pair][:, gsl], ibsb)
                        for ct in range(CT):
                            ops = ps_op.tile([128, 512], F32)
                            nc.tensor.matmul(ops, wo_sb[0][:, ct * 128:(ct + 1) * 128],
                                             onrm[0][:, gsl], start=True, stop=False)
                            nc.tensor.matmul(ops, wo_sb[1][:, ct * 128:(ct + 1) * 128],
                                             onrm[1][:, gsl], start=False, stop=True)
                            ot = ostp.tile([128, 512], F32)
                            if ct % 2 == 0:
                                nc.scalar.copy(ot, ops)
                            else:
                                nc.vector.tensor_copy(ot, ops)
                            nc.sync.dma_start(outT_d[ct * 128:(ct + 1) * 128, gsl], ot)

                    for kt in range(NT):
                        qlo, qhi = max(kt - 2, 0), min(kt + 2, NT - 1)
                        span = (qhi - qlo + 1) * 128
                        chunks = ([(0, 384), (384, span - 384)] if span > 512
                                  else [(0, span)])
                        for h in range(HL):
                            ktile = qk_sb[2 + h // 2][(h % 2) * 64:(h % 2) * 64 + 64,
                                                      kt * 128:(kt + 1) * 128]
                            pt = ptp.tile([128, 640], BF16, tag=f"pt{h}")
                            for (c0, w) in chunks:
                                sps = ps_sc.tile([128, 512], F32)
                                qtile = qk_sb[h // 2][
                                    (h % 2) * 64:(h % 2) * 64 + 64,
                                    qlo * 128 + c0: qlo * 128 + c0 + w]
                                nc.tensor.matmul(sps[:, :w], ktile, qtile,
                                                 start=True, stop=True)
                                nc.scalar.activation(pt[:, c0:c0 + w], sps[:, :w],
                                                     AF.Exp)
                            if kt >= 2:
                                nc.vector.tensor_mul(pt[:, 0:128], pt[:, 0:128], mlo)
                            if kt <= NT - 3:
                                nc.vector.tensor_mul(pt[:, span - 128:span],
                                                     pt[:, span - 128:span], mhi)
                            ptiles[(kt, h)] = (pt, qlo)
                        for qg in range(NT // 4):
                            if kt == min(4 * qg + 5, NT - 1):
                                emit_group(qg)
                    if dbg:
                        nc.sync.dma_start(dbg["d_qk0"], qk_sb[0])
                        nc.sync.dma_start(dbg["d_vsb"], v_sb)
                        nc.sync.dma_start(dbg["d_osb0"], oT_sb[0])
                        nc.sync.dma_start(dbg["d_onrm0"], onrm[0])
                        nc.sync.dma_start(dbg["d_inv"], inv_b[:, :])

    nc.compile()
    nc.m = get_hw_module(nc.m)
    return nc


def _rope_cos_sin(n, d):
    inv_freq = 1.0 / (ROPE_BASE ** (np.arange(0, d, 2, dtype=np.float32) / d))
    freqs = np.arange(n, dtype=np.float32)[:, None] * inv_freq[None, :]
    emb = np.concatenate([freqs, freqs], axis=-1)
    return np.cos(emb).astype(np.float32), np.sin(emb).astype(np.float32)


def _prep_inputs(x, w_qkv, w_out, norm_scale):
    cos, sin = _rope_cos_sin(HEADS, HEAD)
    Wf = np.ascontiguousarray(w_qkv.reshape(3, HEADS, HEAD, SIZE)).astype(np.float32)
    d2 = HEAD // 2

    def rot(Wh, h):
        out = np.empty_like(Wh)
        out[:d2] = cos[h, :d2, None] * Wh[:d2] - sin[h, :d2, None] * Wh[d2:]
        out[d2:] = cos[h, d2:, None] * Wh[d2:] + sin[h, d2:, None] * Wh[:d2]
        return out

    scale = np.float32(1.0 / np.sqrt(HEAD))
    Wq = np.stack([rot(Wf[0, h], h) for h in range(HEADS)]) * scale
    Wk = np.stack([rot(Wf[1, h], h) for h in range(HEADS)])
    Wv = Wf[2]
    ns = norm_scale.astype(np.float32)[None, None, :]
    Wq = Wq * ns
    Wk = Wk * ns
    Wv = Wv * ns

    i = np.arange(128)
    mlo = (i[:, None] <= i[None, :]).astype(ml_dtypes.bfloat16)
    mhi = (i[:, None] >= i[None, :]).astype(ml_dtypes.bfloat16)

    in_maps = []
    for core in range(8):
        b, g = divmod(core, 4)
        hs = slice(HL * g, HL * g + HL)
        wq_l = Wq[hs].reshape(HL * HEAD, SIZE)
        wk_l = Wk[hs].reshape(HL * HEAD, SIZE)
        wv_l = Wv[hs].reshape(HL * HEAD, SIZE)
        wqkvT = np.ascontiguousarray(np.concatenate([wq_l, wk_l, wv_l], 0).T)
        woutT = np.ascontiguousarray(
            w_out[:, HL * HEAD * g: HL * HEAD * (g + 1)].T.astype(np.float32))
        in_maps.append({
            "xT": np.ascontiguousarray(x[b].T).astype(ml_dtypes.bfloat16),
            "wqkvT": wqkvT.astype(ml_dtypes.bfloat16),
            "woutT": woutT.astype(ml_dtypes.bfloat16),
            "onescol": np.ones((128, 1), ml_dtypes.bfloat16),
            "onesrow": np.ones((1, 128), np.float32),
            "mlo": mlo,
            "mhi": mhi,
        })
    return in_maps


def _run(in_maps, trace=False, **kw):
    if "nc" not in _cached:
        _cached["nc"] = _build_module()
    return run_bass_kernel_spmd(_cached["nc"], in_maps, core_ids=list(range(8)),
                                trace=trace, **kw)


def kernel(x, mask, w_qkv, w_out, norm_scale):
    x = np.asarray(x)
    in_maps = _prep_inputs(x, np.asarray(w_qkv), np.asarray(w_out),
                           np.asarray(norm_scale))
    res = _run(in_maps)
    out = np.empty((B, S, SIZE), np.float32)
    for b in range(B):
        acc = res.results[b * 4]["outT"].copy()
        for g in range(1, 4):
            acc += res.results[b * 4 + g]["outT"]
        out[b] = acc.T + x[b]
    return out
